# revision 5
# baseline (speedup 1.0000x reference)
"""Trainium2 Bass kernel for nn_DilatedMHCABlock (dilated multi-head self-attention).

Shapes hardcoded for B=4, N=2048, D=1024, H=16, dh=64, K_WIN=8, DILATION=4.

Decomposition: the dilated mask |j-i| <= 32, (j-i) % 4 == 0 splits each batch
sequence into 4 interleaved subsequences of length 512 with plain +-8 banded
attention.  16 subsequences are sharded 2-per-core across 8 NeuronCores ->
no halo, no collectives.

Per-core program (tokens = [sub0(512) | sub1(512)], activations transposed
[feature, token]):
  - V/Q/K/O projections as fp32r GEMMs with contiguous weight-tile DMAs.
  - q and k are BOTH normalized at PSUM-eviction time with one fused DVE
    scalar_tensor_tensor: (psum + bias) * bcast(1/||.||); the norms are
    computed straight from PSUM (ACT Square with bias -> PE ind_h reduce ->
    ACT Sqrt -> DVE reciprocal -> PE ind2 broadcast).  exp then needs no
    per-partition scale.
  - Attention per (subseq s, query-half p, head h): banded score tiles
    [256 keys x 256 queries] -> one [128,512] ACT Exp -> bf16 in-place DVE
    mask multiply -> bf16 AV matmuls with a ones-column producing softmax
    denominators for free.  Edge keys of 4 heads are stacked into one
    [128,256] PSUM tile via tile_position column offsets (4x fewer edge
    exps); a 4x-replicated v-strip keeps AV operand partitions aligned.
  - AV outputs ([65,256] = 64 feats + sums row) evicted once to SBUF staging
    (DVE/ACT alternating), then two SBUF->SBUF DMAs scatter to the head
    accumulator and the sums tile.  Softmax normalization is deferred:
    reciprocal of sums, PE ind16 broadcast, DVE multiply.
  - biases: bq/bk folded into the normalize-evictions, bv folded into
    bo_eff = bo + Wo @ bv on the host, bo_eff applied at O-eviction.
"""
import sys

sys.path.insert(0, "/opt/trn_rl_repo")

import numpy as np

import bass_rust
import concourse.bass as bass
import concourse.mybir as mybir
import concourse.tile as tile

F32 = mybir.dt.float32
F32R = mybir.dt.float32r
BF16 = mybir.dt.bfloat16
AF = mybir.ActivationFunctionType
ALU = mybir.AluOpType
N_CORES = 8


# ---------------------------------------------------------------------------
# walrus wait legalization: at most 1 sync wait per instruction (2 on
# EventSemaphore); split the excess onto standalone EventSemaphore insts.
_wait_counter = [0]


def _legalize_waits(nc):
    f = nc.m.functions[0]
    for blk in f.blocks:
        insts = blk.instructions
        out = []
        changed = False
        for inst in insts:
            si = inst.sync_info
            waits = list(si.on_wait) if si is not None else []
            cap = 2 if isinstance(inst, mybir.InstEventSemaphore) else 1
            if len(waits) > cap:
                extra, keep = waits[:-cap], waits[-cap:]
                for i in range(0, len(extra), 2):
                    es = mybir.InstEventSemaphore(
                        name=f"wait_split_{_wait_counter[0]}", ins=[], outs=[]
                    )
                    _wait_counter[0] += 1
                    es.engine = inst.engine
                    es.sync_info = bass_rust.SyncInfo(
                        on_wait=extra[i : i + 2], on_update=[]
                    )
                    out.append(es)
                si.on_wait = keep
                changed = True
            out.append(inst)
        if changed:
            blk.instructions = out


def _mm(nc, out, lhsT, rhs, **kw):
    if lhsT.dtype == BF16:
        nc.tensor.matmul(out, lhsT, rhs, **kw)
    else:
        nc.tensor.matmul(out, lhsT.bitcast(F32R), rhs.bitcast(F32R), **kw)


# ---------------------------------------------------------------------------
def _build_nc():
    nc = bass.Bass()

    xT_d = nc.declare_dram_parameter("xT", [1024, 1024], BF16, isOutput=False)
    wq_d = nc.declare_dram_parameter("wq4", [128, 8192], BF16, isOutput=False)
    wk_d = nc.declare_dram_parameter("wk4", [128, 8192], BF16, isOutput=False)
    wo_d = nc.declare_dram_parameter("wo4", [128, 8192], F32R, isOutput=False)
    wv_d = nc.declare_dram_parameter("wvT", [1024, 1024], BF16, isOutput=False)
    bq_d = nc.declare_dram_parameter("bq", [1024], F32, isOutput=False)
    bk_d = nc.declare_dram_parameter("bk", [1024], F32, isOutput=False)
    bo_d = nc.declare_dram_parameter("bo", [1024], F32, isOutput=False)
    mbc_d = nc.declare_dram_parameter("m_bc", [128, 512], BF16, isOutput=False)
    me_d = nc.declare_dram_parameter("m_e", [128, 512], BF16, isOutput=False)
    ih_d = nc.declare_dram_parameter("ind_h", [128, 2], F32R, isOutput=False)
    i2_d = nc.declare_dram_parameter("ind2", [2, 128], F32R, isOutput=False)
    or_d = nc.declare_dram_parameter("ones_r", [128, 64], F32R, isOutput=False)
    out_d = nc.declare_dram_parameter("outT", [1024, 1024], F32, isOutput=True)

    with tile.TileContext(nc) as tc, nc.allow_low_precision(
        reason="fp32r matmuls; bf16 exp/V within 2e-2 tolerance"
    ):
        _emit(nc, tc, xT_d, wq_d, wk_d, wo_d, wv_d, bq_d, bk_d, bo_d,
              mbc_d, me_d, ih_d, i2_d, or_d, out_d)

    _legalize_waits(nc)
    return nc


def _emit(nc, tc, xT_d, wq_d, wk_d, wo_d, wv_d, bq_d, bk_d, bo_d,
          mbc_d, me_d, ih_d, i2_d, or_d, out_d):
    from contextlib import ExitStack

    ctx = ExitStack()
    with ctx:
        p_const = ctx.enter_context(tc.tile_pool(name="const", bufs=1))
        p_xT = ctx.enter_context(tc.tile_pool(name="xT", bufs=1))
        p_atb = ctx.enter_context(tc.tile_pool(name="atb", bufs=8))
        p_qkT = ctx.enter_context(tc.tile_pool(name="qkT", bufs=32))
        p_v = ctx.enter_context(tc.tile_pool(name="v", bufs=12))
        p_w = ctx.enter_context(tc.tile_pool(name="wstream", bufs=2))
        p_wv = ctx.enter_context(tc.tile_pool(name="wvstr", bufs=2))
        p_sq = ctx.enter_context(tc.tile_pool(name="sq", bufs=2))
        p_nq = ctx.enter_context(tc.tile_pool(name="nq", bufs=2))
        p_exp = ctx.enter_context(tc.tile_pool(name="exp", bufs=8))
        p_stg = ctx.enter_context(tc.tile_pool(name="stg", bufs=6))
        p_rs = ctx.enter_context(tc.tile_pool(name="rs", bufs=4))
        p_out = ctx.enter_context(tc.tile_pool(name="outst", bufs=4))
        pp = ctx.enter_context(tc.tile_pool(name="pp", bufs=1, space="PSUM"))

        wvt3 = [None, None]
        wvt3[0] = p_wv.tile([128, 8, 512], BF16, tag="wvstr", name="wvt3a")
        nc.sync.dma_start(
            out=wvt3[0],
            in_=wv_d[:, 0:512].rearrange("(j p) c -> p j c", p=128))
        xT3 = p_xT.tile([128, 8, 1024], BF16, tag="xT", name="xT3")
        nc.sync.dma_start(
            out=xT3[:, 0:4, :],
            in_=xT_d[0:512, :].rearrange("(j p) t -> p j t", p=128))
        wvt3[1] = p_wv.tile([128, 8, 512], BF16, tag="wvstr", name="wvt3b")
        nc.sync.dma_start(
            out=wvt3[1],
            in_=wv_d[:, 512:1024].rearrange("(j p) c -> p j c", p=128))
        nc.sync.dma_start(
            out=xT3[:, 4:8, :],
            in_=xT_d[512:1024, :].rearrange("(j p) t -> p j t", p=128))
        xT = [xT3[:, j, :] for j in range(8)]

        # ---- constants -----------------------------------------------------
        bq_sb = p_const.tile([128, 8], F32, tag="bq")
        bk_sb = p_const.tile([128, 8], F32, tag="bk")
        bo_sb = p_const.tile([128, 8], F32, tag="bo")
        nc.sync.dma_start(out=bq_sb, in_=bq_d.rearrange("(i p) -> p i", p=128))
        nc.sync.dma_start(out=bk_sb, in_=bk_d.rearrange("(i p) -> p i", p=128))
        nc.sync.dma_start(out=bo_sb, in_=bo_d.rearrange("(i p) -> p i", p=128))

        m_bc = p_const.tile([128, 512], BF16, tag="m_bc")
        nc.sync.dma_start(out=m_bc, in_=mbc_d[:, :])
        m_e = p_const.tile([128, 512], BF16, tag="m_e")
        nc.sync.dma_start(out=m_e, in_=me_d[:, :])

        ind_h = p_const.tile([128, 2], F32R, tag="ind_h")
        nc.sync.dma_start(out=ind_h, in_=ih_d[:, :])
        ind2 = p_const.tile([2, 128], F32R, tag="ind2")
        nc.sync.dma_start(out=ind2, in_=i2_d[:, :])
        ones_r = p_const.tile([128, 64], F32R, tag="ones_r")
        nc.sync.dma_start(out=ones_r, in_=or_d[:, :])


        # ---- V projection --------------------------------------------------
        # v[g]: [128 tokens, 16*65] bf16; head h at cols [65h, 65h+64),
        # col 65h+64 = ones (softmax denominator via AV matmul).
        v = [None] * 8
        for ghalf in range(2):
            gs = list(range(4 * ghalf, 4 * ghalf + 4))
            for co in range(2):
                psv = {}
                for gi, g in enumerate(gs):
                    tag = "big" if gi < 2 else "bc"
                    psv[g] = pp.tile([128, 512], F32, tag=tag,
                                     bufs=3 if gi < 2 else 2,
                                     name=f"psv{g}_{co}")
                for j in range(8):
                    for g in gs:
                        _mm(nc, psv[g],
                            xT[j][:, 128 * g : 128 * g + 128],
                            wvt3[co][:, j, :], start=(j == 0), stop=(j == 7))
                for g in gs:
                    if v[g] is None:
                        v[g] = p_v.tile([128, 1040], BF16, tag="v", name=f"v{g}")
                        ones_dst = bass.AP(
                            tensor=v[g].tensor, offset=v[g].offset + 64,
                            ap=[[v[g].ap[0][0], 128], [65, 16], [1, 1]],
                        )
                        nc.vector.memset(ones_dst, 1.0)
                    dst = bass.AP(
                        tensor=v[g].tensor,
                        offset=v[g].offset + 65 * 8 * co,
                        ap=[[v[g].ap[0][0], 128], [65, 8], [1, 64]],
                    )
                    nc.scalar.activation(out=dst, in_=psv[g], func=AF.Identity)

        # replicated edge v-strips: vedge[s][p] = 4x copy of the 32-key strip
        # feeding (s, p)'s edge AV, aligned to stack row-groups {0,32,64,96}.
        vedge = [[None, None] for _ in range(2)]
        for s in range(2):
            for p in range(2):
                ge = 4 * s + 2 + 2 * p * 0 if p == 0 else 4 * s + 1
                er0 = 0 if p == 0 else 96
                t = p_v.tile([128, 1040], BF16, tag="v", name=f"vedge{s}{p}")
                for gp in range(4):
                    nc.sync.dma_start(
                        out=t[32 * gp : 32 * gp + 32, :],
                        in_=v[ge][er0 : er0 + 32, :],
                    )
                vedge[s][p] = t

        # ---- Q / K projections with fused normalize-evictions -------------
        qt = [[None, None] for _ in range(8)]
        kt = [[None, None] for _ in range(8)]

        for w_d, bias_sb, dst in ((wq_d, bq_sb, qt), (wk_d, bk_sb, kt)):
            wt4 = None
            for i in range(8):
                if i % 2 == 0:
                    wt4 = p_w.tile([128, 2, 8, 128], BF16, tag="wstream",
                                   name="wt4")
                    nc.sync.dma_start(
                        out=wt4,
                        in_=w_d[:, 1024 * i : 1024 * i + 2048].rearrange(
                            "p (i j o) -> p i j o", i=2, j=8),
                    )
                wt3 = wt4[:, i % 2]
                ps = [pp.tile([128, 512], F32, tag="big", bufs=3,
                              name=f"ps_{i}_{c}") for c in range(2)]
                for j in range(8):
                    for c in range(2):
                        _mm(nc, ps[c], wt3[:, j, :],
                            xT[j][:, 512 * c : 512 * c + 512],
                            start=(j == 0), stop=(j == 7))
                for c in range(2):
                    bcol = bias_sb[:, i : i + 1]
                    sq = p_sq.tile([128, 512], F32R, tag="sq")
                    nc.scalar.activation(out=sq, in_=ps[c], func=AF.Square,
                                         bias=bcol)
                    pn = pp.tile([2, 512], F32, tag="sm", bufs=3, name="pn")
                    _mm(nc, pn, ind_h, sq, start=True, stop=True)
                    nq = p_nq.tile([2, 512], F32R, tag="nq")
                    nc.scalar.activation(out=nq, in_=pn, func=AF.Sqrt)
                    nc.vector.reciprocal(nq, nq)
                    pb = pp.tile([128, 512], F32, tag="bc", bufs=2, name="pb")
                    _mm(nc, pb, ind2, nq, start=True, stop=True)
                    t = p_qkT.tile([128, 512], BF16, tag="qkT",
                                    name=f"QK{0 if dst is qt else 1}_{i}_{c}")
                    if (i + c) % 2 == 0:
                        nc.vector.tensor_scalar_add(t, ps[c], bcol)
                    else:
                        nc.scalar.activation(out=t, in_=ps[c],
                                             func=AF.Identity, bias=bcol)
                    nc.vector.tensor_mul(t, t, pb)
                    dst[i][c] = t

        # ---- attention -----------------------------------------------------
        atb = [p_atb.tile([128, 1024], F32R, tag="atb", name=f"atb{_i}")
               for _i in range(8)]

        def o_block(i, c):
            if i % 2 == 0:
                wt4 = p_w.tile([128, 2, 8, 128], F32R, tag="wstream",
                               name="wt4o")
                o_wt4[c] = wt4
                nc.sync.dma_start(
                    out=wt4,
                    in_=wo_d[:, 1024 * i : 1024 * i + 2048].rearrange(
                        "p (i j o) -> p i j o", i=2, j=8),
                )
            wt3 = o_wt4[c][:, i % 2]
            ps = pp.tile([128, 512], F32, tag="big", bufs=3, name="ps5")
            for j in range(8):
                _mm(nc, ps, wt3[:, j, :],
                    atb[j][:, 512 * c : 512 * c + 512],
                    start=(j == 0), stop=(j == 7))
            ot = p_out.tile([128, 512], F32, tag="ot", name="ot")
            nc.scalar.activation(out=ot, in_=ps, func=AF.Identity,
                                 bias=bo_sb[:, i : i + 1])
            nc.sync.dma_start(
                out=out_d[128 * i : 128 * i + 128, 512 * c : 512 * c + 512],
                in_=ot)

        o_wt4 = {}
        o_next = [0]
        for s in range(2):
            for p in range(2):
                t0l = 256 * p
                g = 4 * s + 2 * p
                er0 = 0 if p == 0 else 96
                ge = (g + 2) if p == 0 else (g - 1)
                for stk in range(4):
                    hs = [4 * stk + u for u in range(4)]
                    # 4 edge-score matmuls stacked into one [128,256] psum
                    pse = pp.tile([128, 256], F32, tag="bc", bufs=2,
                                  name=f"pse{s}{p}{stk}")
                    for u, h in enumerate(hs):
                        hp = 64 * (h % 2)
                        ht = h // 2
                        Q = qt[ht][s][hp : hp + 64, t0l : t0l + 256]
                        kE = kt[ht][s][
                            hp : hp + 64,
                            128 * (ge % 4) + er0 : 128 * (ge % 4) + er0 + 32,
                        ]
                        _mm(nc, pse[32 * u : 32 * u + 32, :], kE, Q,
                            start=True, stop=True, tile_position=(hp, 32 * u))
                    exe = p_exp.tile([128, 256], BF16, tag="expe", bufs=2,
                                     name="exe")
                    nc.scalar.activation(out=exe, in_=pse, func=AF.Exp)
                    nc.vector.tensor_mul(
                        exe, exe, m_e[:, 256 * p : 256 * p + 256])
                    for u, h in enumerate(hs):
                        hp = 64 * (h % 2)
                        ht = h // 2
                        Q = qt[ht][s][hp : hp + 64, t0l : t0l + 256]
                        kB = kt[ht][s][hp : hp + 64, 256 * p : 256 * p + 128]
                        kC = kt[ht][s][
                            hp : hp + 64, 256 * p + 128 : 256 * p + 256]
                        pbc = pp.tile([128, 512], F32, tag="bc", bufs=2,
                                      name="pbc")
                        _mm(nc, pbc[:, 0:256], kB, Q, start=True, stop=True,
                            tile_position=(hp, 0))
                        _mm(nc, pbc[:, 256:512], kC, Q, start=True, stop=True,
                            tile_position=(hp, 0))
                        exb = p_exp.tile([128, 512], BF16, tag="expbc",
                                         bufs=6, name="exb")
                        nc.scalar.activation(out=exb, in_=pbc, func=AF.Exp)
                        nc.gpsimd.tensor_mul(exb, exb, m_bc)
                        po = pp.tile([65, 256], F32, tag="sm", bufs=3,
                                     name="po")
                        _mm(nc, po, v[g][:, 65 * h : 65 * h + 65],
                            exb[:, 0:256], start=True, stop=False)
                        _mm(nc, po, v[g + 1][:, 65 * h : 65 * h + 65],
                            exb[:, 256:512], start=False, stop=False)
                        _mm(nc, po,
                            vedge[s][p][32 * u : 32 * u + 32,
                                        65 * h : 65 * h + 65],
                            exe[32 * u : 32 * u + 32, :],
                            start=False, stop=True, tile_position=(32 * u, 0))
                        # fused softmax normalization: reciprocal of the
                        # sums row, broadcast via 1-contract matmul, then
                        # normalize-evict in one DVE scalar_tensor_tensor.
                        rs = p_rs.tile([65, 256], F32R, tag="rs", name="rs")
                        nc.vector.reciprocal(rs[64:65, :], po[64:65, :])
                        pb = pp.tile([64, 256], F32, tag="big", bufs=3,
                                     name="pbn")
                        _mm(nc, pb, ones_r[64:65, :], rs[64:65, :],
                            start=True, stop=True, tile_position=(64, 0))
                        stg = p_stg.tile([64, 256], F32R, tag="stg",
                                         name="stg")
                        if h % 2 == 0:
                            nc.vector.tensor_copy(out=stg, in_=po[0:64, :])
                        else:
                            nc.scalar.activation(out=stg, in_=po[0:64, :],
                                                 func=AF.Copy)
                        hp2 = 64 * (h % 2)
                        if hp2 == 0:
                            nc.vector.tensor_mul(
                                atb[ht][0:64,
                                        512 * s + t0l : 512 * s + t0l + 256],
                                stg, pb)
                        else:
                            nc.vector.tensor_mul(stg, stg, pb)
                            nc.sync.dma_start(
                                out=atb[ht][64:128,
                                            512 * s + t0l : 512 * s + t0l + 256],
                                in_=stg,
                            )
                    if s == 1 and o_next[0] < 8:
                        o_block(o_next[0], 0)
                        o_next[0] += 1

        # ---- output projection (c=1; c=0 interleaved into attention) -------
        while o_next[0] < 8:
            o_block(o_next[0], 0)
            o_next[0] += 1
        for i in range(8):
            o_block(i, 1)


# ---------------------------------------------------------------------------
def _build_masks_bc():
    j = np.arange(128)[:, None]
    r = np.arange(256)[None, :]
    M_B = (np.abs(j - r) <= 8).astype(np.float32)
    M_C = (np.abs(128 + j - r) <= 8).astype(np.float32)
    return np.concatenate([M_B, M_C], axis=1)


def _build_masks_e():
    r = np.arange(256)[None, :]
    jj = np.arange(32)[:, None]
    E0 = (r >= 248 + jj).astype(np.float32)  # p=0: edge keys 256+jj
    E1 = (r <= jj - 24).astype(np.float32)   # p=1: edge keys 224+jj
    out = np.zeros((128, 512), np.float32)
    for gp in range(4):
        out[32 * gp : 32 * gp + 32, 0:256] = E0
        out[32 * gp : 32 * gp + 32, 256:512] = E1
    return out


def _w4(W):
    # [128, 8192]: w4[p, i*1024 + j*128 + o] = W[128i+o, 128j+p]
    return np.ascontiguousarray(
        W.reshape(8, 128, 8, 128).transpose(3, 0, 2, 1).reshape(128, 8192)
    )


_NC_CACHE = {}


def _get_nc():
    if "nc" not in _NC_CACHE:
        _NC_CACHE["nc"] = _build_nc()
    return _NC_CACHE["nc"]


def _make_in_maps(inputs, n_cores):
    import ml_dtypes

    x = np.asarray(inputs["x"], dtype=np.float32)
    Wq = np.asarray(inputs["Wq"], dtype=np.float32)
    Wk = np.asarray(inputs["Wk"], dtype=np.float32)
    Wv = np.asarray(inputs["Wv"], dtype=np.float32)
    Wo = np.asarray(inputs["Wo"], dtype=np.float32)
    bq = np.asarray(inputs["bq"], dtype=np.float32)
    bk = np.asarray(inputs["bk"], dtype=np.float32)
    bv = np.asarray(inputs["bv"], dtype=np.float32)
    bo = np.asarray(inputs["bo"], dtype=np.float32)

    bf = ml_dtypes.bfloat16
    m_bc = _build_masks_bc().astype(bf)
    m_e = _build_masks_e().astype(bf)
    ind_h = np.zeros((128, 2), np.float32)
    ind_h[0:64, 0] = 1.0
    ind_h[64:128, 1] = 1.0
    ind2 = np.zeros((2, 128), np.float32)
    ind2[0, 0:64] = 1.0
    ind2[1, 64:128] = 1.0
    ones_r = np.ones((128, 64), np.float32)
    wq4 = _w4(Wq).astype(bf)
    wk4 = _w4(Wk).astype(bf)
    wo4 = _w4(Wo)
    wvT = np.ascontiguousarray(Wv.T).astype(bf)
    bo_eff = (bo + Wo @ bv).astype(np.float32)

    in_maps = []
    for core in range(n_cores):
        subs = [2 * core, 2 * core + 1]
        Xc = np.concatenate([x[u // 4, u % 4 :: 4, :] for u in subs], 0)
        xT = np.ascontiguousarray(Xc.T).astype(bf)
        in_maps.append(
            {
                "xT": xT,
                "wq4": wq4,
                "wk4": wk4,
                "wo4": wo4,
                "wvT": wvT,
                "bq": bq,
                "bk": bk,
                "bo": bo_eff,
                "m_bc": m_bc,
                "m_e": m_e,
                "ind_h": ind_h,
                "ind2": ind2,
                "ones_r": ones_r,
            }
        )
    return in_maps


def kernel(x, Wq, bq, Wk, bk, Wv, bv, Wo, bo, _cores=None):
    from concourse.bass_utils import run_bass_kernel_spmd

    x = np.asarray(x, dtype=np.float32)
    B, N, D = x.shape
    n_cores = N_CORES if _cores is None else _cores
    in_maps = _make_in_maps(
        dict(x=x, Wq=Wq, bq=bq, Wk=Wk, bk=bk, Wv=Wv, bv=bv, Wo=Wo, bo=bo), n_cores
    )
    nc = _get_nc()
    res = run_bass_kernel_spmd(nc, in_maps, core_ids=list(range(n_cores)))

    out = np.zeros((B, N, D), np.float32)
    for core in range(n_cores):
        oc = res.results[core]["outT"].T  # [t, o]
        for i, u in enumerate([2 * core, 2 * core + 1]):
            out[u // 4, u % 4 :: 4, :] = oc[512 * i : 512 * (i + 1)]
    return out
